# revision 8
# baseline (speedup 1.0000x reference)
"""CrossAndCompress Trainium2 kernel (fp16 wire, PE-transpose + PE-dot).

Reference computation (per row r of the batch):
    a_r = enc_item[r] . theta_vv        b_r = enc_user[r] . theta_ev
    c_r = enc_item[r] . theta_ve        d_r = enc_user[r] . theta_ee
    v_out[r] = enc_user[r] * a_r + enc_item[r] * b_r + beta_v
    e_out[r] = enc_user[r] * c_r + enc_item[r] * d_r + beta_e

Sharding: pure data parallel — batch dim (16384) split across 8 NeuronCores
(2048 rows each); theta/beta replicated.

Design rationale (from trace iteration):
  - Correctness gate is 2e-2 → 16-bit wire: host casts inputs to fp16, device
    writes fp16, host upcasts. HBM traffic 16.8MB/core (~46us at the ~360GB/s
    per-core DMA roofline) vs 35.7MB fp32 (~97us floor — fp32 can never win).
  - fp32 baseline was DVE-bound (82%): 4 mul-reduce dot passes + 2 output
    passes per tile, ~1.28us each; fp16 does NOT speed these DVE ops up
    (no 2x perf modes for mul-reduce / scalar_tensor_tensor).
  - Dots move to TensorE, which needs transposed operands in SBUF.
    DMA-xbar transposes ride the same 16 DMA queues as HBM traffic (+8.4MB
    → 70us DMA floor: dead end), so PE transposes (fp16 PSUM) + one
    ScalarE PSUM→SBUF copy pass per tile instead.
  - HWDGE DMA triggers cost ~0.65us queue time each → two row-tiles per
    group, one dma-in + one dma-out per group, out-DMA emitted one group
    late so the in-order sync queue never stalls on it.

Per-core pipeline: 8 groups x [2 tiles x 128 rows x 2048 (u|it packed)]:
  - DMA in xt2 [128, 2, 2048] fp16 (row = g*256 + s*128 + p)        [sync]
  - per tile: 16 PE block-transposes -> xps PSUM fp16 [128,16,128]  [PE]
  - per tile: copy xps -> xT SBUF                                   [ACT]
  - per tile: 16 matmuls xT-chunk @ theta-chunk -> dots PSUM [128,4][PE]
    (b = u.t_ev, d = u.t_ee, a = it.t_vv, c = it.t_ve)
  - per tile: dots PSUM->SBUF [128,4]                               [DVE]
  - per tile: p2 = it*b, p4 = it*d (tensor_scalar, per-part scale)  [GPSIMD]
  - per tile: v = u*a + p2, e = u*c + p4 (scalar_tensor_tensor)     [DVE]
  - DMA out xo2 [128, 2, 2, 1024] fp16 = packed [v | e]             [sync]
Budgets/core: DMA ~46us (bound), DVE ~39us, ACT ~32us, PE ~42us,
GpSimd ~2.6us/tile-pair target, sync ~11us.
"""

import numpy as np

B, D = 16384, 1024
N_CORES = 8
ROWS_PER_CORE = B // N_CORES  # 2048
TILE_P = 128
GROUP_T = 1  # row-tiles per group (1 dma-in + 1 dma-out each)
N_GROUPS = ROWS_PER_CORE // (GROUP_T * TILE_P)  # 8
N_CHUNKS = D // TILE_P  # 8

_PROGRAM_CACHE: dict = {}
_IDENT = np.eye(TILE_P, dtype=np.float16)


def _build_program(with_beta: bool):
    import concourse.mybir as mybir
    import concourse.tile as tile
    from concourse import bacc
    f16 = mybir.dt.float16
    f32 = mybir.dt.float32
    OP = mybir.AluOpType
    AF = mybir.ActivationFunctionType

    nc = bacc.Bacc(
        "TRN2",
        target_bir_lowering=False,
        debug=False,
        enable_asserts=False,
        num_devices=N_CORES,
    )

    # xin[g, s, p, 0:D] = enc_user row (g*256+s*128+p); [.., D:2D] = enc_item
    xin_h = nc.dram_tensor(
        "xin", [N_GROUPS, GROUP_T, TILE_P, 2 * D], f16, kind="ExternalInput"
    ).ap()
    # th_pe[p, c, :]: c<8 -> (t_ev, t_ee) chunk c; c>=8 -> (t_vv, t_ve) c-8
    th_h = nc.dram_tensor("th_pe", [TILE_P, 2 * N_CHUNKS, 2], f16,
                          kind="ExternalInput").ap()
    id_h = nc.dram_tensor("ident", [TILE_P, TILE_P], f16,
                          kind="ExternalInput").ap()
    if with_beta:
        be_h = nc.dram_tensor("betas", [TILE_P, 2, D], f16,
                              kind="ExternalInput").ap()
    # xout[g, s, p, 0, :] = v_out row; [.., 1, :] = e_out row
    xout_h = nc.dram_tensor(
        "xout", [N_GROUPS, GROUP_T, TILE_P, 2, D], f16, kind="ExternalOutput"
    ).ap()

    with tile.TileContext(nc) as tc:
        with (
            tc.tile_pool(name="const", bufs=1) as cpool,
            tc.tile_pool(name="io", bufs=3) as io,
            tc.tile_pool(name="xt", bufs=3) as xtp,
            tc.tile_pool(name="out", bufs=3) as outp,
            tc.tile_pool(name="work", bufs=4) as work,
            tc.tile_pool(name="psx", bufs=2, space="PSUM") as psx,
            tc.tile_pool(name="psd", bufs=4, space="PSUM") as psd,
        ):
            ident = cpool.tile([TILE_P, TILE_P], f16, tag="ident")
            th = cpool.tile([TILE_P, 2 * N_CHUNKS, 2], f16, tag="th")
            if with_beta:
                betas = cpool.tile([TILE_P, 2, D], f16, tag="betas")
                nc.sync.dma_start(betas[:], be_h[:, :, :])

            pending_outs = []  # (dram_ap, sbuf_tile) delayed two groups
            for g in range(N_GROUPS):
                xt2 = io.tile([TILE_P, GROUP_T, 2 * D], f16, tag="xt2")
                nc.sync.dma_start(xt2[:], xin_h[g].rearrange("s p f -> p s f"))
                if g == 0:
                    nc.sync.dma_start(ident[:], id_h[:, :])
                    nc.sync.dma_start(th[:], th_h[:, :, :])
                while len(pending_outs) >= 2:
                    nc.sync.dma_start(*pending_outs.pop(0))

                xo2 = outp.tile([TILE_P, GROUP_T, 2, D], f16, tag="xo2")
                for s in range(GROUP_T):
                    u = xt2[:, s, 0:D]
                    it = xt2[:, s, D : 2 * D]

                    # PE block transposes: xps[p, c, j] = xt2[j, s, c*128+p]
                    xps = psx.tile([TILE_P, 2 * N_CHUNKS, TILE_P], f16,
                                   tag="xps")
                    for c in range(2 * N_CHUNKS):
                        nc.tensor.transpose(
                            xps[:, c, :],
                            xt2[:, s, c * TILE_P : (c + 1) * TILE_P],
                            ident[:],
                        )
                    xT = xtp.tile([TILE_P, 2 * N_CHUNKS, TILE_P], f16,
                                  tag="xT")
                    nc.scalar.copy(xT[:], xps[:])

                    # dots[:,0]=b  [:,1]=d  [:,2]=a  [:,3]=c
                    dots_ps = psd.tile([TILE_P, 4], f32, tag="dots_ps")
                    for c in range(N_CHUNKS):
                        nc.tensor.matmul(
                            dots_ps[:, 0:2], xT[:, c, :], th[:, c, :],
                            start=(c == 0), stop=(c == N_CHUNKS - 1),
                        )
                    for c in range(N_CHUNKS):
                        nc.tensor.matmul(
                            dots_ps[:, 2:4], xT[:, N_CHUNKS + c, :],
                            th[:, N_CHUNKS + c, :],
                            start=(c == 0), stop=(c == N_CHUNKS - 1),
                        )
                    dots = work.tile([TILE_P, 4], f32, tag="dots")
                    nc.vector.tensor_copy(dots[:], dots_ps[:])
                    d_b, d_d = dots[:, 0:1], dots[:, 1:2]
                    d_a, d_c = dots[:, 2:3], dots[:, 3:4]

                    # item-scaled products (tensor_scalar runs fp16 2x on
                    # DVE, 0.49us vs 1.28us stt; GpSimd measured 14ns/elem,
                    # unusable). p2 split 768 ACT / 256 DVE to balance queues.
                    SPL = 768
                    p2 = work.tile([TILE_P, D], f16, tag="p2")
                    nc.scalar.activation(p2[:, 0:SPL], it[:, 0:SPL], AF.Copy,
                                         bias=0.0, scale=d_b)
                    nc.vector.tensor_scalar(out=p2[:, SPL:D], in0=it[:, SPL:D],
                                            scalar1=d_b, scalar2=None,
                                            op0=OP.mult)
                    p4 = work.tile([TILE_P, D], f16, tag="p4")
                    nc.vector.tensor_scalar(out=p4[:], in0=it, scalar1=d_d,
                                            scalar2=None, op0=OP.mult)

                    # v = u*a + p2, e = u*c + p4 as tensor_scalar + fp16
                    # tensor_tensor add (both 2x) instead of one 1x stt
                    va = work.tile([TILE_P, D], f16, tag="va")
                    nc.vector.tensor_scalar(out=va[:], in0=u, scalar1=d_a,
                                            scalar2=None, op0=OP.mult)
                    nc.vector.tensor_tensor(out=xo2[:, s, 0, :], in0=va[:],
                                            in1=p2[:], op=OP.add)
                    ea = work.tile([TILE_P, D], f16, tag="ea")
                    nc.vector.tensor_scalar(out=ea[:], in0=u, scalar1=d_c,
                                            scalar2=None, op0=OP.mult)
                    nc.vector.tensor_tensor(out=xo2[:, s, 1, :], in0=ea[:],
                                            in1=p4[:], op=OP.add)
                    if with_beta:
                        nc.vector.tensor_add(
                            xo2[:, s, :, :], xo2[:, s, :, :], betas[:])
                pending_outs.append(
                    (xout_h[g].rearrange("s p o f -> p s o f"), xo2[:]))
            for po in pending_outs:
                nc.sync.dma_start(*po)

    nc.compile()
    return nc


def _get_program(with_beta: bool):
    if with_beta not in _PROGRAM_CACHE:
        _PROGRAM_CACHE[with_beta] = _build_program(with_beta)
    return _PROGRAM_CACHE[with_beta]


def _prep_host_inputs(inputs):
    enc_user = np.asarray(inputs["enc_user"])
    enc_item = np.asarray(inputs["enc_item"])
    assert enc_user.shape == (B, D) and enc_item.shape == (B, D)

    xin = np.empty((B, 2 * D), dtype=np.float16)
    xin[:, :D] = enc_user
    xin[:, D:] = enc_item

    def vec(name):
        return np.asarray(inputs[name], dtype=np.float32).reshape(D)

    t_vv, t_ev = vec("theta_vv"), vec("theta_ev")
    t_ve, t_ee = vec("theta_ve"), vec("theta_ee")
    # th_pe[p, c, k]: c<8 -> u-dots thetas (t_ev, t_ee); c>=8 -> it-dots
    # thetas (t_vv, t_ve); d-index = (c % 8)*128 + p.
    th_pe = np.empty((TILE_P, 2 * N_CHUNKS, 2), dtype=np.float16)
    th_pe[:, :N_CHUNKS, 0] = t_ev.reshape(N_CHUNKS, TILE_P).T
    th_pe[:, :N_CHUNKS, 1] = t_ee.reshape(N_CHUNKS, TILE_P).T
    th_pe[:, N_CHUNKS:, 0] = t_vv.reshape(N_CHUNKS, TILE_P).T
    th_pe[:, N_CHUNKS:, 1] = t_ve.reshape(N_CHUNKS, TILE_P).T

    beta_v, beta_e = vec("beta_v"), vec("beta_e")
    with_beta = bool(np.any(beta_v) or np.any(beta_e))
    betas_b = None
    if with_beta:
        bb = np.stack([beta_v, beta_e]).astype(np.float16)  # [2, D]
        betas_b = np.ascontiguousarray(
            np.broadcast_to(bb[None, :, :], (TILE_P, 2, D))
        )
    return xin, th_pe, betas_b, with_beta


def _make_in_maps(xin, th_pe, betas_b, with_beta):
    in_maps = []
    for c in range(N_CORES):
        rows = slice(c * ROWS_PER_CORE, (c + 1) * ROWS_PER_CORE)
        m = {
            "xin": xin[rows].reshape(N_GROUPS, GROUP_T, TILE_P, 2 * D),
            "th_pe": th_pe,
            "ident": _IDENT,
        }
        if with_beta:
            m["betas"] = betas_b
        in_maps.append(m)
    return in_maps


def run_on_hw(inputs, trace=False):
    """Build/fetch the program, run it SPMD on 8 cores, gather outputs.

    Returns ((v_out, e_out), BassKernelResults).
    """
    import time

    from concourse.bass_utils import run_bass_kernel_spmd

    host = _prep_host_inputs(inputs)
    with_beta = host[-1]
    nc = _get_program(with_beta)
    in_maps = _make_in_maps(*host)
    for attempt in range(3):
        try:
            res = run_bass_kernel_spmd(nc, in_maps, list(range(N_CORES)), trace=trace)
            break
        except Exception:
            if attempt == 2:
                raise
            time.sleep(2.0)
    xout = np.concatenate(
        [np.asarray(res.results[c]["xout"]).reshape(ROWS_PER_CORE, 2, D)
         for c in range(N_CORES)],
        axis=0,
    )
    v = xout[:, 0, :].astype(np.float32)
    e = xout[:, 1, :].astype(np.float32)
    return (v, e), res


def kernel(**inputs):
    (v, e), _ = run_on_hw(inputs, trace=False)
    return v, e


# revision 9
# speedup vs baseline: 1.1889x; 1.1889x over previous
"""CrossAndCompress Trainium2 kernel (fp16 wire, PE-transpose + PE-dot).

Reference computation (per row r of the batch):
    a_r = enc_item[r] . theta_vv        b_r = enc_user[r] . theta_ev
    c_r = enc_item[r] . theta_ve        d_r = enc_user[r] . theta_ee
    v_out[r] = enc_user[r] * a_r + enc_item[r] * b_r + beta_v
    e_out[r] = enc_user[r] * c_r + enc_item[r] * d_r + beta_e

Sharding: pure data parallel — batch dim (16384) split across 8 NeuronCores
(2048 rows each); theta/beta replicated.

Design rationale (from trace iteration):
  - Correctness gate is 2e-2 → 16-bit wire: host casts inputs to fp16, device
    writes fp16, host upcasts. HBM traffic 16.8MB/core (~46us at the ~360GB/s
    per-core DMA roofline) vs 35.7MB fp32 (~97us floor — fp32 can never win).
  - fp32 baseline was DVE-bound (82%): 4 mul-reduce dot passes + 2 output
    passes per tile, ~1.28us each; fp16 does NOT speed these DVE ops up
    (no 2x perf modes for mul-reduce / scalar_tensor_tensor).
  - Dots move to TensorE, which needs transposed operands in SBUF.
    DMA-xbar transposes ride the same 16 DMA queues as HBM traffic (+8.4MB
    → 70us DMA floor: dead end), so PE transposes (fp16 PSUM) + one
    ScalarE PSUM→SBUF copy pass per tile instead.
  - HWDGE DMA triggers cost ~0.65us queue time each → two row-tiles per
    group, one dma-in + one dma-out per group, out-DMA emitted one group
    late so the in-order sync queue never stalls on it.

Per-core pipeline: 8 groups x [2 tiles x 128 rows x 2048 (u|it packed)]:
  - DMA in xt2 [128, 2, 2048] fp16 (row = g*256 + s*128 + p)        [sync]
  - per tile: 16 PE block-transposes -> xps PSUM fp16 [128,16,128]  [PE]
  - per tile: copy xps -> xT SBUF                                   [ACT]
  - per tile: 16 matmuls xT-chunk @ theta-chunk -> dots PSUM [128,4][PE]
    (b = u.t_ev, d = u.t_ee, a = it.t_vv, c = it.t_ve)
  - per tile: dots PSUM->SBUF [128,4]                               [DVE]
  - per tile: p2 = it*b, p4 = it*d (tensor_scalar, per-part scale)  [GPSIMD]
  - per tile: v = u*a + p2, e = u*c + p4 (scalar_tensor_tensor)     [DVE]
  - DMA out xo2 [128, 2, 2, 1024] fp16 = packed [v | e]             [sync]
Budgets/core: DMA ~46us (bound), DVE ~39us, ACT ~32us, PE ~42us,
GpSimd ~2.6us/tile-pair target, sync ~11us.
"""

import numpy as np

B, D = 16384, 1024
N_CORES = 8
ROWS_PER_CORE = B // N_CORES  # 2048
TILE_P = 128
GROUP_T = 2  # row-tiles per group (1 dma-in + 1 dma-out each)
N_GROUPS = ROWS_PER_CORE // (GROUP_T * TILE_P)  # 8
N_CHUNKS = D // TILE_P  # 8

_PROGRAM_CACHE: dict = {}
_IDENT = np.eye(TILE_P, dtype=np.float16)


def _build_program(with_beta: bool):
    import concourse.mybir as mybir
    import concourse.tile as tile
    from concourse import bacc
    f16 = mybir.dt.float16
    f32 = mybir.dt.float32
    OP = mybir.AluOpType
    AF = mybir.ActivationFunctionType

    nc = bacc.Bacc(
        "TRN2",
        target_bir_lowering=False,
        debug=False,
        enable_asserts=False,
        num_devices=N_CORES,
    )

    # xin[g, s, p, 0:D] = enc_user row (g*256+s*128+p); [.., D:2D] = enc_item
    xin_h = nc.dram_tensor(
        "xin", [N_GROUPS, GROUP_T, TILE_P, 2 * D], f16, kind="ExternalInput"
    ).ap()
    # th_pe[p, c, :]: c<8 -> (t_ev, t_ee) chunk c; c>=8 -> (t_vv, t_ve) c-8
    th_h = nc.dram_tensor("th_pe", [TILE_P, 2 * N_CHUNKS, 2], f16,
                          kind="ExternalInput").ap()
    id_h = nc.dram_tensor("ident", [TILE_P, TILE_P], f16,
                          kind="ExternalInput").ap()
    if with_beta:
        be_h = nc.dram_tensor("betas", [TILE_P, 2, D], f16,
                              kind="ExternalInput").ap()
    # xout[g, s, p, 0, :] = v_out row; [.., 1, :] = e_out row
    xout_h = nc.dram_tensor(
        "xout", [N_GROUPS, GROUP_T, TILE_P, 2, D], f16, kind="ExternalOutput"
    ).ap()

    with tile.TileContext(nc) as tc:
        with (
            tc.tile_pool(name="const", bufs=1) as cpool,
            tc.tile_pool(name="io", bufs=3) as io,
            tc.tile_pool(name="xt", bufs=3) as xtp,
            tc.tile_pool(name="out", bufs=3) as outp,
            tc.tile_pool(name="work", bufs=4) as work,
            tc.tile_pool(name="psx", bufs=2, space="PSUM") as psx,
            tc.tile_pool(name="psd", bufs=4, space="PSUM") as psd,
        ):
            ident = cpool.tile([TILE_P, TILE_P], f16, tag="ident")
            th = cpool.tile([TILE_P, 2 * N_CHUNKS, 2], f16, tag="th")
            if with_beta:
                betas = cpool.tile([TILE_P, 2, D], f16, tag="betas")
                nc.sync.dma_start(betas[:], be_h[:, :, :])

            pending_outs = []  # (dram_ap, sbuf_tile) delayed one group
            for g in range(N_GROUPS):
                xt2 = io.tile([TILE_P, GROUP_T, 2 * D], f16, tag="xt2")
                if g == 0:
                    # finer first DMAs: tile 0 compute starts ~1.5us sooner
                    nc.sync.dma_start(xt2[:, 0:1, :],
                                      xin_h[g, 0:1].rearrange("s p f -> p s f"))
                    nc.sync.dma_start(ident[:], id_h[:, :])
                    nc.sync.dma_start(th[:], th_h[:, :, :])
                    nc.sync.dma_start(xt2[:, 1:2, :],
                                      xin_h[g, 1:2].rearrange("s p f -> p s f"))
                else:
                    nc.sync.dma_start(xt2[:],
                                      xin_h[g].rearrange("s p f -> p s f"))
                while len(pending_outs) >= 1:
                    nc.sync.dma_start(*pending_outs.pop(0))

                xo2 = outp.tile([TILE_P, GROUP_T, 2, D], f16, tag="xo2")
                for s in range(GROUP_T):
                    u = xt2[:, s, 0:D]
                    it = xt2[:, s, D : 2 * D]

                    # PE block transposes: xps[p, c, j] = xt2[j, s, c*128+p]
                    xps = psx.tile([TILE_P, 2 * N_CHUNKS, TILE_P], f16,
                                   tag="xps")
                    for c in range(2 * N_CHUNKS):
                        nc.tensor.transpose(
                            xps[:, c, :],
                            xt2[:, s, c * TILE_P : (c + 1) * TILE_P],
                            ident[:],
                        )
                    xT = xtp.tile([TILE_P, 2 * N_CHUNKS, TILE_P], f16,
                                  tag="xT")
                    nc.scalar.copy(xT[:], xps[:])

                    # dots[:,0]=b  [:,1]=d  [:,2]=a  [:,3]=c
                    dots_ps = psd.tile([TILE_P, 4], f32, tag="dots_ps")
                    for c in range(N_CHUNKS):
                        nc.tensor.matmul(
                            dots_ps[:, 0:2], xT[:, c, :], th[:, c, :],
                            start=(c == 0), stop=(c == N_CHUNKS - 1),
                        )
                    for c in range(N_CHUNKS):
                        nc.tensor.matmul(
                            dots_ps[:, 2:4], xT[:, N_CHUNKS + c, :],
                            th[:, N_CHUNKS + c, :],
                            start=(c == 0), stop=(c == N_CHUNKS - 1),
                        )
                    dots = work.tile([TILE_P, 4], f32, tag="dots")
                    nc.vector.tensor_copy(dots[:], dots_ps[:])
                    d_b, d_d = dots[:, 0:1], dots[:, 1:2]
                    d_a, d_c = dots[:, 2:3], dots[:, 3:4]

                    # item-scaled products (tensor_scalar runs fp16 2x on
                    # DVE, 0.49us vs 1.28us stt; GpSimd measured 14ns/elem,
                    # unusable). p2 split 768 ACT / 256 DVE to balance queues.
                    SPL = 768
                    p2 = work.tile([TILE_P, D], f16, tag="p2")
                    nc.scalar.activation(p2[:, 0:SPL], it[:, 0:SPL], AF.Copy,
                                         bias=0.0, scale=d_b)
                    nc.vector.tensor_scalar(out=p2[:, SPL:D], in0=it[:, SPL:D],
                                            scalar1=d_b, scalar2=None,
                                            op0=OP.mult)
                    p4 = work.tile([TILE_P, D], f16, tag="p4")
                    nc.vector.tensor_scalar(out=p4[:], in0=it, scalar1=d_d,
                                            scalar2=None, op0=OP.mult)

                    # v = u*a + p2, e = u*c + p4 as tensor_scalar + fp16
                    # tensor_tensor add (both 2x) instead of one 1x stt
                    va = work.tile([TILE_P, D], f16, tag="va")
                    nc.vector.tensor_scalar(out=va[:], in0=u, scalar1=d_a,
                                            scalar2=None, op0=OP.mult)
                    nc.vector.tensor_tensor(out=xo2[:, s, 0, :], in0=va[:],
                                            in1=p2[:], op=OP.add)
                    ea = work.tile([TILE_P, D], f16, tag="ea")
                    nc.vector.tensor_scalar(out=ea[:], in0=u, scalar1=d_c,
                                            scalar2=None, op0=OP.mult)
                    nc.vector.tensor_tensor(out=xo2[:, s, 1, :], in0=ea[:],
                                            in1=p4[:], op=OP.add)
                    if with_beta:
                        nc.vector.tensor_add(
                            xo2[:, s, :, :], xo2[:, s, :, :], betas[:])
                if g == N_GROUPS - 1:
                    # finer last DMAs: shorter tail
                    nc.sync.dma_start(
                        xout_h[g, 0:1].rearrange("s p o f -> p s o f"),
                        xo2[:, 0:1])
                    nc.sync.dma_start(
                        xout_h[g, 1:2].rearrange("s p o f -> p s o f"),
                        xo2[:, 1:2])
                else:
                    pending_outs.append(
                        (xout_h[g].rearrange("s p o f -> p s o f"), xo2[:]))
            for po in pending_outs:
                nc.sync.dma_start(*po)

    nc.compile()
    return nc


def _get_program(with_beta: bool):
    if with_beta not in _PROGRAM_CACHE:
        _PROGRAM_CACHE[with_beta] = _build_program(with_beta)
    return _PROGRAM_CACHE[with_beta]


def _prep_host_inputs(inputs):
    enc_user = np.asarray(inputs["enc_user"])
    enc_item = np.asarray(inputs["enc_item"])
    assert enc_user.shape == (B, D) and enc_item.shape == (B, D)

    xin = np.empty((B, 2 * D), dtype=np.float16)
    xin[:, :D] = enc_user
    xin[:, D:] = enc_item

    def vec(name):
        return np.asarray(inputs[name], dtype=np.float32).reshape(D)

    t_vv, t_ev = vec("theta_vv"), vec("theta_ev")
    t_ve, t_ee = vec("theta_ve"), vec("theta_ee")
    # th_pe[p, c, k]: c<8 -> u-dots thetas (t_ev, t_ee); c>=8 -> it-dots
    # thetas (t_vv, t_ve); d-index = (c % 8)*128 + p.
    th_pe = np.empty((TILE_P, 2 * N_CHUNKS, 2), dtype=np.float16)
    th_pe[:, :N_CHUNKS, 0] = t_ev.reshape(N_CHUNKS, TILE_P).T
    th_pe[:, :N_CHUNKS, 1] = t_ee.reshape(N_CHUNKS, TILE_P).T
    th_pe[:, N_CHUNKS:, 0] = t_vv.reshape(N_CHUNKS, TILE_P).T
    th_pe[:, N_CHUNKS:, 1] = t_ve.reshape(N_CHUNKS, TILE_P).T

    beta_v, beta_e = vec("beta_v"), vec("beta_e")
    with_beta = bool(np.any(beta_v) or np.any(beta_e))
    betas_b = None
    if with_beta:
        bb = np.stack([beta_v, beta_e]).astype(np.float16)  # [2, D]
        betas_b = np.ascontiguousarray(
            np.broadcast_to(bb[None, :, :], (TILE_P, 2, D))
        )
    return xin, th_pe, betas_b, with_beta


def _make_in_maps(xin, th_pe, betas_b, with_beta):
    in_maps = []
    for c in range(N_CORES):
        rows = slice(c * ROWS_PER_CORE, (c + 1) * ROWS_PER_CORE)
        m = {
            "xin": xin[rows].reshape(N_GROUPS, GROUP_T, TILE_P, 2 * D),
            "th_pe": th_pe,
            "ident": _IDENT,
        }
        if with_beta:
            m["betas"] = betas_b
        in_maps.append(m)
    return in_maps


def run_on_hw(inputs, trace=False):
    """Build/fetch the program, run it SPMD on 8 cores, gather outputs.

    Returns ((v_out, e_out), BassKernelResults).
    """
    import time

    from concourse.bass_utils import run_bass_kernel_spmd

    host = _prep_host_inputs(inputs)
    with_beta = host[-1]
    nc = _get_program(with_beta)
    in_maps = _make_in_maps(*host)
    for attempt in range(3):
        try:
            res = run_bass_kernel_spmd(nc, in_maps, list(range(N_CORES)), trace=trace)
            break
        except Exception:
            if attempt == 2:
                raise
            time.sleep(2.0)
    xout = np.concatenate(
        [np.asarray(res.results[c]["xout"]).reshape(ROWS_PER_CORE, 2, D)
         for c in range(N_CORES)],
        axis=0,
    )
    v = xout[:, 0, :].astype(np.float32)
    e = xout[:, 1, :].astype(np.float32)
    return (v, e), res


def kernel(**inputs):
    (v, e), _ = run_on_hw(inputs, trace=False)
    return v, e


# revision 10
# speedup vs baseline: 1.2120x; 1.0194x over previous
"""CrossAndCompress Trainium2 kernel (fp16 wire, PE-transpose + PE-dot).

Reference computation (per row r of the batch):
    a_r = enc_item[r] . theta_vv        b_r = enc_user[r] . theta_ev
    c_r = enc_item[r] . theta_ve        d_r = enc_user[r] . theta_ee
    v_out[r] = enc_user[r] * a_r + enc_item[r] * b_r + beta_v
    e_out[r] = enc_user[r] * c_r + enc_item[r] * d_r + beta_e

Sharding: pure data parallel — batch dim (16384) split across 8 NeuronCores
(2048 rows each); theta/beta replicated.

Design rationale (from trace iteration):
  - Correctness gate is 2e-2 → 16-bit wire: host casts inputs to fp16, device
    writes fp16, host upcasts. HBM traffic 16.8MB/core (~46us at the ~360GB/s
    per-core DMA roofline) vs 35.7MB fp32 (~97us floor — fp32 can never win).
  - fp32 baseline was DVE-bound (82%): 4 mul-reduce dot passes + 2 output
    passes per tile, ~1.28us each; fp16 does NOT speed these DVE ops up
    (no 2x perf modes for mul-reduce / scalar_tensor_tensor).
  - Dots move to TensorE, which needs transposed operands in SBUF.
    DMA-xbar transposes ride the same 16 DMA queues as HBM traffic (+8.4MB
    → 70us DMA floor: dead end), so PE transposes (fp16 PSUM) + one
    ScalarE PSUM→SBUF copy pass per tile instead.
  - HWDGE DMA triggers cost ~0.65us queue time each → two row-tiles per
    group, one dma-in + one dma-out per group, out-DMA emitted one group
    late so the in-order sync queue never stalls on it.

Per-core pipeline: 8 groups x [2 tiles x 128 rows x 2048 (u|it packed)]:
  - DMA in xt2 [128, 2, 2048] fp16 (row = g*256 + s*128 + p)        [sync]
  - per tile: 16 PE block-transposes -> xps PSUM fp16 [128,16,128]  [PE]
  - per tile: copy xps -> xT SBUF                                   [ACT]
  - per tile: 16 matmuls xT-chunk @ theta-chunk -> dots PSUM [128,4][PE]
    (b = u.t_ev, d = u.t_ee, a = it.t_vv, c = it.t_ve)
  - per tile: dots PSUM->SBUF [128,4]                               [DVE]
  - per tile: p2 = it*b, p4 = it*d (tensor_scalar, per-part scale)  [GPSIMD]
  - per tile: v = u*a + p2, e = u*c + p4 (scalar_tensor_tensor)     [DVE]
  - DMA out xo2 [128, 2, 2, 1024] fp16 = packed [v | e]             [sync]
Budgets/core: DMA ~46us (bound), DVE ~39us, ACT ~32us, PE ~42us,
GpSimd ~2.6us/tile-pair target, sync ~11us.
"""

import numpy as np

B, D = 16384, 1024
N_CORES = 8
ROWS_PER_CORE = B // N_CORES  # 2048
TILE_P = 128
GROUP_T = 2  # row-tiles per group (1 dma-in + 1 dma-out each)
N_GROUPS = ROWS_PER_CORE // (GROUP_T * TILE_P)  # 8
N_CHUNKS = D // TILE_P  # 8

_PROGRAM_CACHE: dict = {}
_IDENT = np.eye(TILE_P, dtype=np.float16)


def _build_program(with_beta: bool):
    import concourse.mybir as mybir
    import concourse.tile as tile
    from concourse import bacc
    f16 = mybir.dt.float16
    f32 = mybir.dt.float32
    OP = mybir.AluOpType
    AF = mybir.ActivationFunctionType

    nc = bacc.Bacc(
        "TRN2",
        target_bir_lowering=False,
        debug=False,
        enable_asserts=False,
        num_devices=N_CORES,
    )

    # xin[g, s, p, 0:D] = enc_user row (g*256+s*128+p); [.., D:2D] = enc_item
    xin_h = nc.dram_tensor(
        "xin", [N_GROUPS, GROUP_T, TILE_P, 2 * D], f16, kind="ExternalInput"
    ).ap()
    # th_pe[p, c, :]: c<8 -> (t_ev, t_ee) chunk c; c>=8 -> (t_vv, t_ve) c-8
    th_h = nc.dram_tensor("th_pe", [TILE_P, 2 * N_CHUNKS, 2], f16,
                          kind="ExternalInput").ap()
    id_h = nc.dram_tensor("ident", [TILE_P, TILE_P], f16,
                          kind="ExternalInput").ap()
    if with_beta:
        be_h = nc.dram_tensor("betas", [TILE_P, 2, D], f16,
                              kind="ExternalInput").ap()
    # xout[g, s, p, 0, :] = v_out row; [.., 1, :] = e_out row
    xout_h = nc.dram_tensor(
        "xout", [N_GROUPS, GROUP_T, TILE_P, 2, D], f16, kind="ExternalOutput"
    ).ap()

    with tile.TileContext(nc) as tc:
        with (
            tc.tile_pool(name="const", bufs=1) as cpool,
            tc.tile_pool(name="io", bufs=3) as io,
            tc.tile_pool(name="xt", bufs=3) as xtp,
            tc.tile_pool(name="out", bufs=3) as outp,
            tc.tile_pool(name="work", bufs=4) as work,
            tc.tile_pool(name="psx", bufs=2, space="PSUM") as psx,
            tc.tile_pool(name="psd", bufs=4, space="PSUM") as psd,
        ):
            ident = cpool.tile([TILE_P, TILE_P], f16, tag="ident")
            th = cpool.tile([TILE_P, 2 * N_CHUNKS, 2], f16, tag="th")
            if with_beta:
                betas = cpool.tile([TILE_P, 2, D], f16, tag="betas")
                nc.sync.dma_start(betas[:], be_h[:, :, :])

            pending_outs = []  # (dram_ap, sbuf_tile) delayed one group
            for g in range(N_GROUPS):
                xt2 = io.tile([TILE_P, GROUP_T, 2 * D], f16, tag="xt2")
                if g == 0:
                    # consts first (tiny), then finer first data DMAs so
                    # tile 0's transposes start as early as possible
                    nc.sync.dma_start(ident[:], id_h[:, :])
                    nc.sync.dma_start(th[:], th_h[:, :, :])
                    nc.sync.dma_start(xt2[:, 0:1, :],
                                      xin_h[g, 0:1].rearrange("s p f -> p s f"))
                    nc.sync.dma_start(xt2[:, 1:2, :],
                                      xin_h[g, 1:2].rearrange("s p f -> p s f"))
                else:
                    nc.sync.dma_start(xt2[:],
                                      xin_h[g].rearrange("s p f -> p s f"))
                while len(pending_outs) >= 1:
                    nc.sync.dma_start(*pending_outs.pop(0))

                xo2 = outp.tile([TILE_P, GROUP_T, 2, D], f16, tag="xo2")
                for s in range(GROUP_T):
                    u = xt2[:, s, 0:D]
                    it = xt2[:, s, D : 2 * D]

                    # PE block transposes: xps[p, c, j] = xt2[j, s, c*128+p]
                    xps = psx.tile([TILE_P, 2 * N_CHUNKS, TILE_P], f16,
                                   tag="xps")
                    for c in range(2 * N_CHUNKS):
                        nc.tensor.transpose(
                            xps[:, c, :],
                            xt2[:, s, c * TILE_P : (c + 1) * TILE_P],
                            ident[:],
                        )
                    xT = xtp.tile([TILE_P, 2 * N_CHUNKS, TILE_P], f16,
                                  tag="xT")
                    nc.scalar.copy(xT[:], xps[:])

                    # dots[:,0]=b  [:,1]=d  [:,2]=a  [:,3]=c
                    dots_ps = psd.tile([TILE_P, 4], f32, tag="dots_ps")
                    for c in range(N_CHUNKS):
                        nc.tensor.matmul(
                            dots_ps[:, 0:2], xT[:, c, :], th[:, c, :],
                            start=(c == 0), stop=(c == N_CHUNKS - 1),
                        )
                    for c in range(N_CHUNKS):
                        nc.tensor.matmul(
                            dots_ps[:, 2:4], xT[:, N_CHUNKS + c, :],
                            th[:, N_CHUNKS + c, :],
                            start=(c == 0), stop=(c == N_CHUNKS - 1),
                        )
                    dots = work.tile([TILE_P, 4], f32, tag="dots")
                    nc.vector.tensor_copy(dots[:], dots_ps[:])
                    d_b, d_d = dots[:, 0:1], dots[:, 1:2]
                    d_a, d_c = dots[:, 2:3], dots[:, 3:4]

                    # item-scaled products (tensor_scalar runs fp16 2x on
                    # DVE, 0.49us vs 1.28us stt; GpSimd measured 14ns/elem,
                    # unusable). p2 split 768 ACT / 256 DVE to balance queues.
                    SPL = 896
                    p2 = work.tile([TILE_P, D], f16, tag="p2")
                    nc.scalar.activation(p2[:, 0:SPL], it[:, 0:SPL], AF.Copy,
                                         bias=0.0, scale=d_b)
                    nc.vector.tensor_scalar(out=p2[:, SPL:D], in0=it[:, SPL:D],
                                            scalar1=d_b, scalar2=None,
                                            op0=OP.mult)
                    p4 = work.tile([TILE_P, D], f16, tag="p4")
                    nc.vector.tensor_scalar(out=p4[:], in0=it, scalar1=d_d,
                                            scalar2=None, op0=OP.mult)

                    # v = u*a + p2, e = u*c + p4 as tensor_scalar + fp16
                    # tensor_tensor add (both 2x) instead of one 1x stt
                    va = work.tile([TILE_P, D], f16, tag="va")
                    nc.vector.tensor_scalar(out=va[:], in0=u, scalar1=d_a,
                                            scalar2=None, op0=OP.mult)
                    nc.vector.tensor_tensor(out=xo2[:, s, 0, :], in0=va[:],
                                            in1=p2[:], op=OP.add)
                    ea = work.tile([TILE_P, D], f16, tag="ea")
                    nc.vector.tensor_scalar(out=ea[:], in0=u, scalar1=d_c,
                                            scalar2=None, op0=OP.mult)
                    nc.vector.tensor_tensor(out=xo2[:, s, 1, :], in0=ea[:],
                                            in1=p4[:], op=OP.add)
                    if with_beta:
                        nc.vector.tensor_add(
                            xo2[:, s, :, :], xo2[:, s, :, :], betas[:])
                if g == N_GROUPS - 1:
                    # finer last DMAs: shorter tail
                    nc.sync.dma_start(
                        xout_h[g, 0:1].rearrange("s p o f -> p s o f"),
                        xo2[:, 0:1])
                    nc.sync.dma_start(
                        xout_h[g, 1:2].rearrange("s p o f -> p s o f"),
                        xo2[:, 1:2])
                else:
                    pending_outs.append(
                        (xout_h[g].rearrange("s p o f -> p s o f"), xo2[:]))
            for po in pending_outs:
                nc.sync.dma_start(*po)

    nc.compile()
    return nc


def _get_program(with_beta: bool):
    if with_beta not in _PROGRAM_CACHE:
        _PROGRAM_CACHE[with_beta] = _build_program(with_beta)
    return _PROGRAM_CACHE[with_beta]


def _prep_host_inputs(inputs):
    enc_user = np.asarray(inputs["enc_user"])
    enc_item = np.asarray(inputs["enc_item"])
    assert enc_user.shape == (B, D) and enc_item.shape == (B, D)

    xin = np.empty((B, 2 * D), dtype=np.float16)
    xin[:, :D] = enc_user
    xin[:, D:] = enc_item

    def vec(name):
        return np.asarray(inputs[name], dtype=np.float32).reshape(D)

    t_vv, t_ev = vec("theta_vv"), vec("theta_ev")
    t_ve, t_ee = vec("theta_ve"), vec("theta_ee")
    # th_pe[p, c, k]: c<8 -> u-dots thetas (t_ev, t_ee); c>=8 -> it-dots
    # thetas (t_vv, t_ve); d-index = (c % 8)*128 + p.
    th_pe = np.empty((TILE_P, 2 * N_CHUNKS, 2), dtype=np.float16)
    th_pe[:, :N_CHUNKS, 0] = t_ev.reshape(N_CHUNKS, TILE_P).T
    th_pe[:, :N_CHUNKS, 1] = t_ee.reshape(N_CHUNKS, TILE_P).T
    th_pe[:, N_CHUNKS:, 0] = t_vv.reshape(N_CHUNKS, TILE_P).T
    th_pe[:, N_CHUNKS:, 1] = t_ve.reshape(N_CHUNKS, TILE_P).T

    beta_v, beta_e = vec("beta_v"), vec("beta_e")
    with_beta = bool(np.any(beta_v) or np.any(beta_e))
    betas_b = None
    if with_beta:
        bb = np.stack([beta_v, beta_e]).astype(np.float16)  # [2, D]
        betas_b = np.ascontiguousarray(
            np.broadcast_to(bb[None, :, :], (TILE_P, 2, D))
        )
    return xin, th_pe, betas_b, with_beta


def _make_in_maps(xin, th_pe, betas_b, with_beta):
    in_maps = []
    for c in range(N_CORES):
        rows = slice(c * ROWS_PER_CORE, (c + 1) * ROWS_PER_CORE)
        m = {
            "xin": xin[rows].reshape(N_GROUPS, GROUP_T, TILE_P, 2 * D),
            "th_pe": th_pe,
            "ident": _IDENT,
        }
        if with_beta:
            m["betas"] = betas_b
        in_maps.append(m)
    return in_maps


def run_on_hw(inputs, trace=False):
    """Build/fetch the program, run it SPMD on 8 cores, gather outputs.

    Returns ((v_out, e_out), BassKernelResults).
    """
    import time

    from concourse.bass_utils import run_bass_kernel_spmd

    host = _prep_host_inputs(inputs)
    with_beta = host[-1]
    nc = _get_program(with_beta)
    in_maps = _make_in_maps(*host)
    for attempt in range(3):
        try:
            res = run_bass_kernel_spmd(nc, in_maps, list(range(N_CORES)), trace=trace)
            break
        except Exception:
            if attempt == 2:
                raise
            time.sleep(2.0)
    xout = np.concatenate(
        [np.asarray(res.results[c]["xout"]).reshape(ROWS_PER_CORE, 2, D)
         for c in range(N_CORES)],
        axis=0,
    )
    v = xout[:, 0, :].astype(np.float32)
    e = xout[:, 1, :].astype(np.float32)
    return (v, e), res


def kernel(**inputs):
    (v, e), _ = run_on_hw(inputs, trace=False)
    return v, e
